# revision 3
# baseline (speedup 1.0000x reference)
"""Distributed causal multi-head attention for Trainium2 (8 NeuronCores).

Problem (nn_Attention): B=2, T=2048, D=2048, H=16 heads, d_head=128.
  q/k/v = x @ {q,k,v}_out; per-head causal softmax attention; out = ctx @ w_out.

Sharding: batch (2) x head-group (4 heads each) -> 8 cores. Each core computes
its batch's attention for its 4 heads plus the partial output projection
(w_out row-sharded); the host sums the 4 partials per batch (all-reduce) and
stacks batches.

Per-core kernel (all matmuls fp32r, moving free dim 512):
  phase 1: V  = x @ wv   (natural layout, all 4 heads at once)
  phase 2: per head: Q^T, K^T projections; then flash-style causal attention
           with scores kept transposed (tk on partitions):
             S^T chunk = K^T_chunk.T @ Q^T_tile        (PE)
             P^T = exp(S^T / sqrt(dh))                 (ACT, PSUM->SBUF)
             diagonal chunks masked via precomputed 0/1 mask    (DVE)
             C^T += V_chunk.T @ P^T ; Z += ones.T @ P^T (PE, PSUM accum)
             C^T_norm = C^T * 1/Z -> DRAM scratch      (DVE)
  phase 3: out[tq, :] += sum_h C_h @ wo_h  (PSUM accum over heads)

Host passes x^T (per batch) so the contraction dim D is on partitions
everywhere; no on-device transposes needed anywhere.
"""

import math

import numpy as np

import concourse.bacc as bacc
import concourse.mybir as mybir
import concourse.tile as tile
from concourse.bass_utils import run_bass_kernel_spmd

# ---- problem constants (hardcoded; self-contained) ----
B = 2
T = 2048
D = 2048
H_PER = 4            # heads per core
DH = 128             # head dim
GCOLS = H_PER * DH   # 512 columns per head-group
P = 128
KC = D // P          # 16 contraction chunks
TT = 512             # t tile (matmul moving free dim)
NTT = T // TT        # 4
NTCH = T // P        # 16 t chunks
DOT = 512            # output-dim tile
NDOT = D // DOT      # 4

F32 = mybir.dt.float32
F32R = mybir.dt.float32r
SCALE = 1.0 / math.sqrt(float(DH))

_CACHE = {}


def _build():
    nc = bacc.Bacc("TRN2", target_bir_lowering=False, debug=False)
    xT_d = nc.dram_tensor("xT", (D, T), F32R, kind="ExternalInput")
    wq_d = nc.dram_tensor("wq", (D, GCOLS), F32R, kind="ExternalInput")
    wk_d = nc.dram_tensor("wk", (D, GCOLS), F32R, kind="ExternalInput")
    wv_d = nc.dram_tensor("wv", (D, GCOLS), F32R, kind="ExternalInput")
    wo_d = nc.dram_tensor("wo", (GCOLS, D), F32R, kind="ExternalInput")
    ones_d = nc.dram_tensor("ones", (P, P), F32R, kind="ExternalInput")
    out_d = nc.dram_tensor("out", (T, D), F32, kind="ExternalOutput")

    xT_r = xT_d.ap().rearrange("(c p) t -> p c t", p=P)      # (128, 16, 2048)
    wq_r = wq_d.ap().rearrange("(c p) g -> p c g", p=P)      # (128, 16, 512)
    wk_r = wk_d.ap().rearrange("(c p) g -> p c g", p=P)
    wv_r = wv_d.ap().rearrange("(c p) g -> p c g", p=P)
    wo_r = wo_d.ap().rearrange("(h p) n -> p h n", p=P)      # (128, 4, 2048)
    out_r = out_d.ap()

    with tile.TileContext(nc) as tc:
        with (
            tc.tile_pool(name="const", bufs=1) as const_pool,
            tc.tile_pool(name="big", bufs=1) as big_pool,
            tc.tile_pool(name="vp", bufs=1) as v_pool,
            tc.tile_pool(name="work", bufs=5) as work_pool,
            tc.tile_pool(name="dramp", bufs=1, space="DRAM") as dram_pool,
        ):
            # ---- constants ----
            ones_t = const_pool.tile([P, P], F32R, tag="ones", name="ones_t")
            nc.sync.dma_start(ones_t[:], ones_d[:])
            # maskbig[p, u] = 1.0 iff u >= p + (TT - P); diagonal-chunk mask c
            # (c = 0..3) is the slice [:, (TT-P) - P*c :][:TT].
            maskw = TT + (H_PER - 1) * P   # 896
            maskbig = const_pool.tile([P, maskw], F32, tag="mask", name="maskbig")
            nc.gpsimd.memset(maskbig[:], 1.0)
            nc.gpsimd.affine_select(
                out=maskbig[:],
                in_=maskbig[:],
                compare_op=mybir.AluOpType.is_ge,
                fill=0.0,
                base=-(TT - P),
                pattern=[[1, maskw]],
                channel_multiplier=-1,
            )

            def mask_ap(c):
                off = (TT - P) - P * c
                return maskbig[:, off : off + TT]

            # ---- resident x^T ----
            xT_t = big_pool.tile([P, KC, T], F32R, tag="big", name="xT_t")
            for c in range(KC):
                nc.sync.dma_start(xT_t[:, c], xT_r[:, c])

            v_all = v_pool.tile([P, NTCH, GCOLS], F32R, tag="v", name="v_all")
            cT_dram = dram_pool.tile([P, H_PER, T], F32R, tag="ct", name="cT_dram")

            # ---------- phase 1: V = x @ wv (natural layout, all heads) ----------
            with (
                tc.tile_pool(name="wvp", bufs=3) as wv_pool,
                tc.tile_pool(name="psV", bufs=1, space="PSUM") as psV,
            ):
                for half in range(2):
                    wv_tiles = []
                    for k in range(KC):
                        wvk = wv_pool.tile(
                            [P, GCOLS], F32R, tag="wv", name=f"wv_{half}_{k}"
                        )
                        nc.sync.dma_start(wvk[:], wv_r[:, k])
                        wv_tiles.append(wvk)
                    pss = []
                    for ti in range(8):
                        ps = psV.tile(
                            [P, GCOLS], F32, tag=f"pv{ti}", name=f"psv_{half}_{ti}"
                        )
                        pss.append(ps)
                    for k in range(KC):
                        for ti in range(8):
                            tch = half * 8 + ti
                            nc.tensor.matmul(
                                pss[ti][:],
                                xT_t[:, k, tch * P : (tch + 1) * P],
                                wv_tiles[k][:],
                                start=(k == 0),
                                stop=(k == KC - 1),
                            )
                    for ti in range(8):
                        tch = half * 8 + ti
                        nc.vector.tensor_copy(out=v_all[:, tch], in_=pss[ti][:])

            # ---------- phase 2: per-head Q^T/K^T projection + attention ----------
            with (
                tc.tile_pool(name="wqk", bufs=1) as wqk_pool,
                tc.tile_pool(name="qk", bufs=1) as qk_pool,
                tc.tile_pool(name="psQK", bufs=2, space="PSUM") as psQK,
                tc.tile_pool(name="psS", bufs=3, space="PSUM") as psS,
                tc.tile_pool(name="psC", bufs=2, space="PSUM") as psC,
                tc.tile_pool(name="psZ", bufs=1, space="PSUM") as psZ,
            ):
                for h in range(H_PER):
                    hs = slice(h * DH, (h + 1) * DH)
                    qT_t = qk_pool.tile([P, T], F32R, tag="qT", name=f"qT_{h}")
                    kT_t = qk_pool.tile([P, T], F32R, tag="kT", name=f"kT_{h}")
                    wq_t = wqk_pool.tile([P, KC, DH], F32R, tag="wq", name=f"wq_{h}")
                    wk_t = wqk_pool.tile([P, KC, DH], F32R, tag="wk", name=f"wk_{h}")
                    nc.sync.dma_start(wq_t[:], wq_r[:, :, hs])
                    nc.sync.dma_start(wk_t[:], wk_r[:, :, hs])
                    for w_t, dst, nm in ((wq_t, qT_t, "q"), (wk_t, kT_t, "k")):
                        for ti in range(NTT):
                            tsl = slice(ti * TT, (ti + 1) * TT)
                            ps = psQK.tile(
                                [P, TT], F32, tag="qk", name=f"ps{nm}_{h}_{ti}"
                            )
                            for k in range(KC):
                                nc.tensor.matmul(
                                    ps[:],
                                    w_t[:, k],
                                    xT_t[:, k, tsl],
                                    start=(k == 0),
                                    stop=(k == KC - 1),
                                )
                            nc.vector.tensor_copy(out=dst[:, tsl], in_=ps[:])

                    # attention for head h
                    for ti in range(NTT):
                        tsl = slice(ti * TT, (ti + 1) * TT)
                        nch = H_PER * (ti + 1)   # active tk chunks (causal)
                        cT_ps = psC.tile([P, TT], F32, tag="c", name=f"c_{h}_{ti}")
                        z_ps = psZ.tile([P, TT], F32, tag="z", name=f"z_{h}_{ti}")

                        def post(ci, s_ps, h=h, ti=ti, nch=nch, cT_ps=cT_ps, z_ps=z_ps):
                            tslq = slice(ti * TT, (ti + 1) * TT)
                            p_sb = work_pool.tile(
                                [P, TT], F32R, tag="w", name=f"p_{h}_{ti}_{ci}"
                            )
                            nc.scalar.activation(
                                p_sb[:],
                                s_ps[:],
                                mybir.ActivationFunctionType.Exp,
                                scale=SCALE,
                            )
                            rel = ci - (nch - H_PER)
                            if rel >= 0:
                                nc.vector.tensor_mul(
                                    out=p_sb[:], in0=p_sb[:], in1=mask_ap(rel)
                                )
                            nc.tensor.matmul(
                                cT_ps[:],
                                v_all[:, ci, hs],
                                p_sb[:],
                                start=(ci == 0),
                                stop=(ci == nch - 1),
                            )
                            nc.tensor.matmul(
                                z_ps[:],
                                ones_t[:],
                                p_sb[:],
                                start=(ci == 0),
                                stop=(ci == nch - 1),
                            )

                        pending = None
                        for ci in range(nch):
                            s_ps = psS.tile(
                                [P, TT], F32, tag="s", name=f"s_{h}_{ti}_{ci}"
                            )
                            nc.tensor.matmul(
                                s_ps[:],
                                kT_t[:, ci * P : (ci + 1) * P],
                                qT_t[:, tsl],
                                start=True,
                                stop=True,
                            )
                            if pending is not None:
                                post(*pending)
                            pending = (ci, s_ps)
                        post(*pending)

                        recip = work_pool.tile([P, TT], F32, tag="w", name=f"r_{h}_{ti}")
                        nc.vector.reciprocal(recip[:], z_ps[:])
                        cst = work_pool.tile([P, TT], F32R, tag="w", name=f"cn_{h}_{ti}")
                        nc.vector.tensor_mul(out=cst[:], in0=cT_ps[:], in1=recip[:])
                        nc.sync.dma_start(cT_dram[:, h, tsl], cst[:])

            # ---------- phase 3: out = concat_h(C_h) @ wo ----------
            with (
                tc.tile_pool(name="cch", bufs=1) as cch_pool,
                tc.tile_pool(name="psO", bufs=4, space="PSUM") as psO,
            ):
                wo_t = big_pool.tile([P, H_PER, D], F32R, tag="big", name="wo_t")
                for do in range(NDOT):
                    dsl = slice(do * DOT, (do + 1) * DOT)
                    nc.sync.dma_start(wo_t[:, :, dsl], wo_r[:, :, dsl])
                cfull = cch_pool.tile([P, H_PER, T], F32R, tag="cf", name="cfull")
                for h in range(H_PER):
                    nc.sync.dma_start(cfull[:, h], cT_dram[:, h])
                for tq in range(NTCH):
                    for do in range(NDOT):
                        dsl = slice(do * DOT, (do + 1) * DOT)
                        ps = psO.tile([P, DOT], F32, tag="o", name=f"po_{tq}_{do}")
                        for h in range(H_PER):
                            nc.tensor.matmul(
                                ps[:],
                                cfull[:, h, tq * P : (tq + 1) * P],
                                wo_t[:, h, dsl],
                                start=(h == 0),
                                stop=(h == H_PER - 1),
                            )
                        ost = work_pool.tile(
                            [P, DOT], F32, tag="w", name=f"ost_{tq}_{do}"
                        )
                        nc.vector.tensor_copy(out=ost[:], in_=ps[:])
                        nc.sync.dma_start(
                            out_r[tq * P : (tq + 1) * P, dsl], ost[:]
                        )

    nc.compile()
    return nc


def _get_nc():
    if "nc" not in _CACHE:
        _CACHE["nc"] = _build()
    return _CACHE["nc"]


def kernel(**inputs) -> np.ndarray:
    x = np.ascontiguousarray(np.asarray(inputs["x"], dtype=np.float32))
    q_out = np.ascontiguousarray(np.asarray(inputs["q_out"], dtype=np.float32))
    k_out = np.ascontiguousarray(np.asarray(inputs["k_out"], dtype=np.float32))
    v_out = np.ascontiguousarray(np.asarray(inputs["v_out"], dtype=np.float32))
    w_out = np.ascontiguousarray(np.asarray(inputs["w_out"], dtype=np.float32))

    nc = _get_nc()
    ones = np.ones((P, P), dtype=np.float32)
    in_maps = []
    for b in range(B):
        xT = np.ascontiguousarray(x[b].T)
        for g in range(4):  # head groups
            cols = slice(g * GCOLS, (g + 1) * GCOLS)
            in_maps.append(
                {
                    "xT": xT,
                    "wq": np.ascontiguousarray(q_out[:, cols]),
                    "wk": np.ascontiguousarray(k_out[:, cols]),
                    "wv": np.ascontiguousarray(v_out[:, cols]),
                    "wo": np.ascontiguousarray(w_out[cols, :]),
                    "ones": ones,
                }
            )

    res = run_bass_kernel_spmd(nc, in_maps, core_ids=list(range(8)))
    outs = [res.results[c]["out"] for c in range(8)]
    full = np.stack(
        [
            outs[0] + outs[1] + outs[2] + outs[3],
            outs[4] + outs[5] + outs[6] + outs[7],
        ]
    )
    return full.astype(np.float32)


# revision 15
# speedup vs baseline: 18580.2800x; 18580.2800x over previous
"""Distributed causal multi-head attention for Trainium2 (8 NeuronCores).

Problem (nn_Attention): B=2, T=2048, D=2048, H=16 heads, d_head=128.
  q/k/v = x @ {q,k,v}_out; per-head causal softmax attention; out = ctx @ w_out.

Sharding: batch (2) x head-group (4 heads each) -> 8 cores. Each core computes
its batch's attention for its 4 heads plus the partial output projection
(w_out row-sharded); the host sums the 4 partials per batch (all-reduce) and
stacks batches.

Per-core kernel (all matmuls fp32r, moving free dim 512):
  phase 1: V  = x @ wv   (natural layout, all 4 heads at once)
  phase 2: per head: Q^T, K^T projections; then flash-style causal attention
           with scores kept transposed (tk on partitions):
             S^T chunk = K^T_chunk.T @ Q^T_tile        (PE)
             P^T = exp(S^T / sqrt(dh))                 (ACT, PSUM->SBUF)
             diagonal chunks masked via precomputed 0/1 mask    (DVE)
             C^T += V_chunk.T @ P^T ; Z += ones.T @ P^T (PE, PSUM accum)
             C^T_norm = C^T * 1/Z -> DRAM scratch      (DVE)
  phase 3: out[tq, :] += sum_h C_h @ wo_h  (PSUM accum over heads)

Host passes x^T (per batch) so the contraction dim D is on partitions
everywhere; no on-device transposes needed anywhere.
"""

import math

import numpy as np

import concourse.bacc as bacc
import concourse.mybir as mybir
import concourse.tile as tile
from concourse.bass_utils import run_bass_kernel_spmd

# ---- problem constants (hardcoded; self-contained) ----
B = 2
T = 2048
D = 2048
H_PER = 4            # heads per core
DH = 128             # head dim
GCOLS = H_PER * DH   # 512 columns per head-group
P = 128
KC = D // P          # 16 contraction chunks
TT = 512             # t tile (matmul moving free dim)
NTT = T // TT        # 4
NTCH = T // P        # 16 t chunks
DOT = 512            # output-dim tile
NDOT = D // DOT      # 4

F32 = mybir.dt.float32
F32R = mybir.dt.float32r
SCALE = 1.0 / math.sqrt(float(DH))

_CACHE = {}


def _build(n_repeat=1):
    nc = bacc.Bacc("TRN2", target_bir_lowering=False, debug=False)
    xT_d = nc.dram_tensor("xT", (D, T), F32R, kind="ExternalInput")
    wq_d = nc.dram_tensor("wq", (D, GCOLS), F32R, kind="ExternalInput")
    wk_d = nc.dram_tensor("wk", (D, GCOLS), F32R, kind="ExternalInput")
    wv_d = nc.dram_tensor("wv", (D, GCOLS), F32R, kind="ExternalInput")
    wo_d = nc.dram_tensor("wo", (GCOLS, D), F32R, kind="ExternalInput")
    ones_d = nc.dram_tensor("ones", (P, P), F32R, kind="ExternalInput")
    out_d = nc.dram_tensor("out", (T, D), F32, kind="ExternalOutput")

    xT_r = xT_d.ap().rearrange("(c p) t -> p c t", p=P)      # (128, 16, 2048)
    wq_r = wq_d.ap().rearrange("(c p) g -> p c g", p=P)      # (128, 16, 512)
    wk_r = wk_d.ap().rearrange("(c p) g -> p c g", p=P)
    wv_r = wv_d.ap().rearrange("(c p) g -> p c g", p=P)
    wo_r = wo_d.ap().rearrange("(h p) n -> p h n", p=P)      # (128, 4, 2048)
    out_r = out_d.ap()

    with tile.TileContext(nc) as tc:
        with (
            tc.tile_pool(name="const", bufs=1) as const_pool,
            tc.tile_pool(name="big", bufs=1) as big_pool,
            tc.tile_pool(name="vp", bufs=1) as v_pool,
            tc.tile_pool(name="work", bufs=5) as work_pool,
            tc.tile_pool(name="dramp", bufs=1, space="DRAM") as dram_pool,
        ):
            # ---- constants ----
            ones_t = const_pool.tile([P, P], F32R, tag="ones", name="ones_t")
            nc.sync.dma_start(ones_t[:], ones_d[:])
            # maskbig[p, u] = 1.0 iff u >= p + (TT - P); diagonal-chunk mask c
            # (c = 0..3) is the slice [:, (TT-P) - P*c :][:TT].
            maskw = TT + (H_PER - 1) * P   # 896
            maskbig = const_pool.tile([P, maskw], F32, tag="mask", name="maskbig")
            nc.gpsimd.memset(maskbig[:], 1.0)
            nc.gpsimd.affine_select(
                out=maskbig[:],
                in_=maskbig[:],
                compare_op=mybir.AluOpType.is_ge,
                fill=0.0,
                base=-(TT - P),
                pattern=[[1, maskw]],
                channel_multiplier=-1,
            )

            def mask_ap(c):
                off = (TT - P) - P * c
                return maskbig[:, off : off + TT]

            # warm the ACT exp table during phase 1 (LoadActFuncSet is ~1.3us
            # and otherwise stalls the first real exp)
            actwarm = const_pool.tile([P, 1], F32, tag="actwarm", name="actwarm")
            nc.scalar.activation(
                actwarm[:], maskbig[:, 0:1], mybir.ActivationFunctionType.Exp
            )

            for rep in range(n_repeat):
                _emit_body(
                    nc, tc, rep, big_pool, v_pool, work_pool, dram_pool,
                    ones_t, mask_ap, xT_r, wq_r, wk_r, wv_r, wo_r, out_r,
                )

    nc.compile()
    return nc


def _emit_body(nc, tc, rep, big_pool, v_pool, work_pool, dram_pool,
               ones_t, mask_ap, xT_r, wq_r, wk_r, wv_r, wo_r, out_r):
    R = f"r{rep}_"

    # ---- resident x^T ----
    xT_t = big_pool.tile([P, KC, T], F32R, tag="big", name=f"{R}xT_t")

    v_all = v_pool.tile([P, NTCH, GCOLS], F32R, tag="v", name=f"{R}v_all")
    cT_drams = [
        dram_pool.tile([P, T], F32R, tag=f"ct{h}", name=f"{R}cT_dram{h}")
        for h in range(H_PER)
    ]

    # ---------- phase 1: V = x @ wv (natural layout, all heads) ----------
    # K split in halves with partial sums so each wv chunk is read from HBM
    # exactly once (phase 1 is DMA-bound; xT alone is 16MB).
    # wqk pool opens BEFORE wvp so its stack addresses don't overlap wvp's
    # released zone (otherwise head-0's weight DMAs wait on the last V matmul).
    KH = KC // 2
    wqk_pool = tc.alloc_tile_pool(name=f"{R}wqk", bufs=1)
    # psQK pre-allocated below psV on the PSUM stack so head-0's projection
    # matmuls don't inherit a released-zone wait on the V-phase accumulators
    psQK = tc.alloc_tile_pool(name=f"{R}psQK", bufs=2, space="PSUM")
    tgroups = [range(0, 6), range(6, 12), range(12, 16)]
    with (
        tc.tile_pool(name=f"{R}wvp", bufs=1) as wv_pool,
        tc.tile_pool(name=f"{R}psV", bufs=1, space="PSUM") as psV,
    ):
        for khalf in range(2):
            wvh = wv_pool.tile([P, KH, GCOLS], F32R, tag="wv", name=f"{R}wv_{khalf}")
            for kk in range(KH):
                # interleave wv + xT chunk loads so phase-1 matmuls can start
                # as soon as the first chunks land (not after the full 16MB)
                nc.sync.dma_start(wvh[:, kk], wv_r[:, khalf * KH + kk])
                nc.sync.dma_start(xT_t[:, khalf * KH + kk], xT_r[:, khalf * KH + kk])
            for gi, tgroup in enumerate(tgroups):
                pss = {}
                for ti, tch in enumerate(tgroup):
                    pss[tch] = psV.tile(
                        [P, GCOLS], F32, tag=f"pv{ti}", name=f"{R}psv_{khalf}_{gi}_{ti}"
                    )
                for kk in range(KH):
                    k = khalf * KH + kk
                    for tch in tgroup:
                        nc.tensor.matmul(
                            pss[tch][:],
                            xT_t[:, k, tch * P : (tch + 1) * P],
                            wvh[:, kk],
                            start=(kk == 0),
                            stop=(kk == KH - 1),
                        )
                for tch in tgroup:
                    if khalf == 0:
                        nc.vector.tensor_copy(out=v_all[:, tch], in_=pss[tch][:])
                    else:
                        nc.vector.tensor_add(
                            out=v_all[:, tch], in0=v_all[:, tch], in1=pss[tch][:]
                        )

    # ---------- phase 2: per-head Q^T/K^T projection + attention ----------
    with (
        tc.tile_pool(name=f"{R}qk", bufs=1) as qk_pool,
        tc.tile_pool(name=f"{R}psS", bufs=3, space="PSUM") as psS,
        tc.tile_pool(name=f"{R}psC", bufs=2, space="PSUM") as psC,
        tc.tile_pool(name=f"{R}psZ", bufs=1, space="PSUM") as psZ,
    ):
        for h in range(H_PER):
            hs = slice(h * DH, (h + 1) * DH)
            qT_t = qk_pool.tile([P, T], F32R, tag="qT", name=f"{R}qT_{h}")
            kT_t = qk_pool.tile([P, T], F32R, tag="kT", name=f"{R}kT_{h}")
            wq_t = wqk_pool.tile([P, KC, DH], F32R, tag="wq", name=f"{R}wq_{h}")
            wk_t = wqk_pool.tile([P, KC, DH], F32R, tag="wk", name=f"{R}wk_{h}")
            # SWDGE (gpsimd) queues: don't sit behind phase-1/attention bulk
            # HWDGE traffic, so head-0's weights land during phase 1
            nc.gpsimd.dma_start(wq_t[:], wq_r[:, :, hs])
            nc.gpsimd.dma_start(wk_t[:], wk_r[:, :, hs])
            for w_t, dst, nm in ((wq_t, qT_t, "q"), (wk_t, kT_t, "k")):
                for ti in range(NTT):
                    tsl = slice(ti * TT, (ti + 1) * TT)
                    ps = psQK.tile([P, TT], F32, tag="qk", name=f"{R}ps{nm}_{h}_{ti}")
                    for k in range(KC):
                        nc.tensor.matmul(
                            ps[:],
                            w_t[:, k],
                            xT_t[:, k, tsl],
                            start=(k == 0),
                            stop=(k == KC - 1),
                        )
                    nc.vector.tensor_copy(out=dst[:, tsl], in_=ps[:])

            # attention for head h
            for ti in range(NTT):
                tsl = slice(ti * TT, (ti + 1) * TT)
                nch = H_PER * (ti + 1)   # active tk chunks (causal)
                cT_ps = psC.tile([P, TT], F32, tag="c", name=f"{R}c_{h}_{ti}")
                z_ps = psZ.tile([P, TT], F32, tag="z", name=f"{R}z_{h}_{ti}")

                def post(ci, s_ps, h=h, ti=ti, nch=nch, cT_ps=cT_ps, z_ps=z_ps,
                         hs=hs):
                    p_sb = work_pool.tile(
                        [P, TT], F32R, tag="w", name=f"{R}p_{h}_{ti}_{ci}"
                    )
                    nc.scalar.activation(
                        p_sb[:],
                        s_ps[:],
                        mybir.ActivationFunctionType.Exp,
                        scale=SCALE,
                    )
                    rel = ci - (nch - H_PER)
                    if rel >= 0:
                        nc.vector.tensor_mul(
                            out=p_sb[:], in0=p_sb[:], in1=mask_ap(rel)
                        )
                    nc.tensor.matmul(
                        cT_ps[:],
                        v_all[:, ci, hs],
                        p_sb[:],
                        start=(ci == 0),
                        stop=(ci == nch - 1),
                    )
                    nc.tensor.matmul(
                        z_ps[:],
                        ones_t[:],
                        p_sb[:],
                        start=(ci == 0),
                        stop=(ci == nch - 1),
                    )

                pending = None
                for ci in range(nch):
                    s_ps = psS.tile([P, TT], F32, tag="s", name=f"{R}s_{h}_{ti}_{ci}")
                    nc.tensor.matmul(
                        s_ps[:],
                        kT_t[:, ci * P : (ci + 1) * P],
                        qT_t[:, tsl],
                        start=True,
                        stop=True,
                    )
                    if pending is not None:
                        post(*pending)
                    pending = (ci, s_ps)
                post(*pending)

                recip = work_pool.tile([P, TT], F32, tag="w", name=f"{R}rc_{h}_{ti}")
                nc.vector.reciprocal(recip[:], z_ps[:])
                cst = work_pool.tile([P, TT], F32R, tag="w", name=f"{R}cn_{h}_{ti}")
                nc.vector.tensor_mul(out=cst[:], in0=cT_ps[:], in1=recip[:])
                nc.sync.dma_start(cT_drams[h][:, tsl], cst[:])

    wqk_pool.release()
    psQK.release()

    # ---------- phase 3: out = concat_h(C_h) @ wo ----------
    # C^T chunks streamed from scratch in tq-groups of 4 (double-buffered)
    # instead of one bulk 4MB load, so the first outproj matmuls start as
    # soon as the first 256KB group lands after head 3 finishes.
    TQG = 4                       # tq chunks per group
    NGR = NTCH // TQG             # 4 groups
    with (
        tc.tile_pool(name=f"{R}cch", bufs=2) as cch_pool,
        tc.tile_pool(name=f"{R}psO", bufs=4, space="PSUM") as psO,
    ):
        wo_t = big_pool.tile([P, H_PER, D], F32R, tag="big", name=f"{R}wo_t")
        for do in range(NDOT):
            dsl = slice(do * DOT, (do + 1) * DOT)
            nc.sync.dma_start(wo_t[:, :, dsl], wo_r[:, :, dsl])
        for g in range(NGR):
            gsl = slice(g * TQG * P, (g + 1) * TQG * P)
            cg = cch_pool.tile([P, H_PER, TQG * P], F32R, tag="cg", name=f"{R}cg_{g}")
            for h in range(H_PER):
                nc.sync.dma_start(cg[:, h], cT_drams[h][:, gsl])
            for tqi in range(TQG):
                tq = g * TQG + tqi
                for do in range(NDOT):
                    dsl = slice(do * DOT, (do + 1) * DOT)
                    ps = psO.tile([P, DOT], F32, tag="o", name=f"{R}po_{tq}_{do}")
                    for h in range(H_PER):
                        nc.tensor.matmul(
                            ps[:],
                            cg[:, h, tqi * P : (tqi + 1) * P],
                            wo_t[:, h, dsl],
                            start=(h == 0),
                            stop=(h == H_PER - 1),
                        )
                    ost = work_pool.tile([P, DOT], F32, tag="w", name=f"{R}ost_{tq}_{do}")
                    nc.vector.tensor_copy(out=ost[:], in_=ps[:])
                    nc.sync.dma_start(out_r[tq * P : (tq + 1) * P, dsl], ost[:])


def _get_nc(n_repeat=1):
    key = f"nc{n_repeat}"
    if key not in _CACHE:
        _CACHE[key] = _build(n_repeat)
    return _CACHE[key]


def make_in_maps(x, q_out, k_out, v_out, w_out):
    ones = np.ones((P, P), dtype=np.float32)
    in_maps = []
    for b in range(B):
        xT = np.ascontiguousarray(x[b].T)
        for g in range(4):  # head groups
            cols = slice(g * GCOLS, (g + 1) * GCOLS)
            in_maps.append(
                {
                    "xT": xT,
                    "wq": np.ascontiguousarray(q_out[:, cols]),
                    "wk": np.ascontiguousarray(k_out[:, cols]),
                    "wv": np.ascontiguousarray(v_out[:, cols]),
                    "wo": np.ascontiguousarray(w_out[cols, :]),
                    "ones": ones,
                }
            )
    return in_maps


def kernel(**inputs) -> np.ndarray:
    x = np.ascontiguousarray(np.asarray(inputs["x"], dtype=np.float32))
    q_out = np.ascontiguousarray(np.asarray(inputs["q_out"], dtype=np.float32))
    k_out = np.ascontiguousarray(np.asarray(inputs["k_out"], dtype=np.float32))
    v_out = np.ascontiguousarray(np.asarray(inputs["v_out"], dtype=np.float32))
    w_out = np.ascontiguousarray(np.asarray(inputs["w_out"], dtype=np.float32))

    nc = _get_nc()
    in_maps = make_in_maps(x, q_out, k_out, v_out, w_out)
    res = run_bass_kernel_spmd(nc, in_maps, core_ids=list(range(8)))
    outs = [res.results[c]["out"] for c in range(8)]
    full = np.stack(
        [
            outs[0] + outs[1] + outs[2] + outs[3],
            outs[4] + outs[5] + outs[6] + outs[7],
        ]
    )
    return full.astype(np.float32)
